# revision 5
# baseline (speedup 1.0000x reference)
"""Adaptive bilinear sampler (nn_AdaptiveSam) on 8 Trainium2 NeuronCores.

The reference module reduces exactly to per-(batch, channel) bilinear
resampling of x at coordinates derived from stride_h/stride_w: the 3x3
dilation taps are integer offsets (so they share one fractional weight) and
the fixed weight tensor is the identity center tap, which kills every tap
except u=v=1.  out[b,c,i,j] = lerp2d(x[b,c], ch[b,i], cw[b,j]).

Device strategy (pure data parallel, 2 batches per core):
  stage 1: dma_gather(transpose=True) pulls column pairs (gj, gj+1) from a
           host-pre-transposed bf16 copy of x; the transposed write leaves
           h on (partition, q) so the intermediate lands in HBM scratch
           already oriented for the second gather.
  stage 2: dma_gather pulls row pairs (fi, fi+1) of the scratch; a fused
           bilinear blend on the vector engine produces the f32 output,
           DMA'd out as 896 B runs.
All sampling indices/weights are computed on host (tiny metadata) and passed
as extra input tensors, so the compiled graph is input-independent.
"""

import os
import sys

sys.path.insert(0, "/opt/trn_rl_repo")
os.environ.setdefault("MYCRO_LOCAL_CACHE", "1")

import numpy as np
import ml_dtypes

import concourse.bass as bass
import concourse.bacc as bacc
import concourse.mybir as mybir
import concourse.tile as tile
from concourse.bass_utils import run_bass_kernel_spmd

N_CORES = 8
B_FULL, C, H, W = 16, 3, 1024, 1024
OUT = 224
NB = B_FULL // N_CORES          # batches per core = 2
NPAD = 768                      # per-batch padded gather-slot count (672 used)
NVAL = C * OUT                  # 672
N_IDX1 = NB * NPAD              # 1536 stage-1 gather slots
SCR_ROW = 2 * NPAD              # scratch row length (lo | hi), elems
OUT2 = 256                      # stage-2 gather slots (224 real + 32 pad)

_PROGRAM = None


def _build_program():
    nc = bacc.Bacc(None)
    bf16 = mybir.dt.bfloat16
    f32 = mybir.dt.float32
    i16 = mybir.dt.int16

    xt = nc.declare_dram_parameter("xt", [NB * C * W, H], bf16, isOutput=False)
    meta_i = nc.declare_dram_parameter(
        "meta_i", [128, N_IDX1 // 16 + NB * (OUT2 // 16)], i16, isOutput=False
    )
    meta_f = nc.declare_dram_parameter(
        "meta_f", [128, NB * NVAL + NB * 2], f32, isOutput=False
    )
    out = nc.declare_dram_parameter("out", [NB, C, OUT, OUT], f32, isOutput=True)

    scratch = nc.dram_tensor("scratch", [NB, H, SCR_ROW], bf16)

    IDX2_OFF = N_IDX1 // 16            # 96
    GB_W = NB * NVAL                   # 1344

    with tile.TileContext(nc) as tc, tc.tile_pool(name="main", bufs=1) as pool:
        mi = pool.tile([128, N_IDX1 // 16 + NB * (OUT2 // 16)], i16, name="mi")
        nc.sync.dma_start(mi[:], meta_i[:])
        mf = pool.tile([128, GB_W + NB * 2], f32, name="mf")
        nc.sync.dma_start(mf[:], meta_f[:])

        # stage 1: gather column pairs, one gather per batch (SWDGE ring cap).
        in1 = bass.AP(xt, 0, [[H, NB * C * W - 1], [1, 2 * H]])
        g1s = []
        for b in range(NB):
            g1 = pool.tile([128, 16, NPAD], bf16, name=f"g1_{b}")
            g1s.append(g1)
            nc.gpsimd.dma_gather(
                g1[:],
                in1,
                mi[:, b * (NPAD // 16) : (b + 1) * (NPAD // 16)],
                num_idxs=NPAD,
                num_idxs_reg=NPAD,
                elem_size=2 * H,
                elem_step=H,
                transpose=True,
            )

        # stage 1b: spill to scratch, rows h = q*128 + p.
        for b in range(NB):
            for half in range(2):  # 0 = lo column, 1 = hi column
                src = g1s[b][:, half * 8 : (half + 1) * 8, :]
                dst = bass.AP(
                    scratch,
                    b * H * SCR_ROW + half * NPAD,
                    [[SCR_ROW, 128], [128 * SCR_ROW, 8], [1, NPAD]],
                )
                nc.sync.dma_start(dst, src)

        # stage 2 + blend + out, per batch.
        for b in range(NB):
            g2 = pool.tile([128, 2, 2 * SCR_ROW], bf16, name=f"g2_{b}")
            in2 = bass.AP(scratch, b * H * SCR_ROW, [[SCR_ROW, H - 1], [1, 2 * SCR_ROW]])
            nc.gpsimd.dma_gather(
                g2[:],
                in2,
                mi[:, IDX2_OFF + b * (OUT2 // 16) : IDX2_OFF + (b + 1) * (OUT2 // 16)],
                num_idxs=OUT2,
                num_idxs_reg=OUT2,
                elem_size=2 * SCR_ROW,
                elem_step=SCR_ROW,
                transpose=False,
            )

            a00 = g2[:, :, 0:NVAL]
            a01 = g2[:, :, NPAD : NPAD + NVAL]
            a10 = g2[:, :, SCR_ROW : SCR_ROW + NVAL]
            a11 = g2[:, :, SCR_ROW + NPAD : SCR_ROW + NPAD + NVAL]

            d0 = pool.tile([128, 2, NVAL], f32, name=f"d0_{b}")
            nc.vector.tensor_tensor(out=d0[:], in0=a10, in1=a00, op=mybir.AluOpType.subtract)
            d1 = pool.tile([128, 2, NVAL], f32, name=f"d1_{b}")
            nc.vector.tensor_tensor(out=d1[:], in0=a11, in1=a01, op=mybir.AluOpType.subtract)

            c0 = pool.tile([128, 2, NVAL], f32, name=f"c0_{b}")
            c1 = pool.tile([128, 2, NVAL], f32, name=f"c1_{b}")
            for k in range(2):
                fcol = mf[:, GB_W + b * 2 + k : GB_W + b * 2 + k + 1]
                nc.vector.scalar_tensor_tensor(
                    out=c0[:, k, :], in0=d0[:, k, :], scalar=fcol, in1=a00[:, k, :],
                    op0=mybir.AluOpType.mult, op1=mybir.AluOpType.add,
                )
                nc.vector.scalar_tensor_tensor(
                    out=c1[:, k, :], in0=d1[:, k, :], scalar=fcol, in1=a01[:, k, :],
                    op0=mybir.AluOpType.mult, op1=mybir.AluOpType.add,
                )

            d2 = pool.tile([128, 2, NVAL], f32, name=f"d2_{b}")
            nc.vector.tensor_tensor(out=d2[:], in0=c1[:], in1=c0[:], op=mybir.AluOpType.subtract)
            res = pool.tile([128, 2, NVAL], f32, name=f"res_{b}")
            gbv = mf[:, b * NVAL : (b + 1) * NVAL]
            for k in range(2):
                m = pool.tile([128, NVAL], f32, name=f"m_{b}_{k}")
                nc.vector.tensor_tensor(out=m[:], in0=d2[:, k, :], in1=gbv, op=mybir.AluOpType.mult)
                nc.vector.tensor_tensor(out=res[:, k, :], in0=m[:], in1=c0[:, k, :], op=mybir.AluOpType.add)

            for k, npart in ((0, 128), (1, OUT - 128)):
                src = res[0:npart, k, :].rearrange("p (c j) -> p c j", c=C)
                dst = bass.AP(
                    out,
                    b * C * OUT * OUT + k * 128 * OUT,
                    [[OUT, npart], [OUT * OUT, C], [1, OUT]],
                )
                nc.sync.dma_start(dst, src)

    nc.compile()
    return nc


def _get_program():
    global _PROGRAM
    if _PROGRAM is None:
        _PROGRAM = _build_program()
    return _PROGRAM


def _wrap16(vals):
    """Pack a flat index list into the [128, n/16] SWDGE wrapped layout."""
    n = len(vals)
    assert n % 16 == 0
    arr = np.asarray(vals, np.int16).reshape(n // 16, 16).T  # [16, n/16]
    return np.tile(arr, (8, 1))  # replicate across the 8 Q7 core groups


def make_in_maps(x, stride_h, stride_w):
    """Host-side metadata + sharding. Returns per-core input maps."""
    ch = (stride_h + 1.0) * (H - 1) * 0.5
    cw = (stride_w + 1.0) * (W - 1) * 0.5
    fi = np.floor(ch).astype(np.int64)
    fv = (ch - fi).astype(np.float32)
    gj = np.floor(cw).astype(np.int64)
    gv = (cw - gj).astype(np.float32)

    xb = x.astype(ml_dtypes.bfloat16)

    in_maps = []
    for core in range(N_CORES):
        b0 = core * NB
        xt = np.empty((NB * C * W, H), ml_dtypes.bfloat16)
        for b in range(NB):
            for c in range(C):
                xt[(b * C + c) * W : (b * C + c + 1) * W] = xb[b0 + b, c].T

        idx1 = np.zeros(N_IDX1, np.int64)
        for b in range(NB):
            m = np.arange(NPAD)
            cc = np.minimum(m // OUT, C - 1)
            jj = m % OUT
            val = b * C * W + cc * W + gj[b0 + b][np.minimum(jj, OUT - 1)]
            val[m >= NVAL] = b * C * W  # harmless pad reads
            idx1[b * NPAD : (b + 1) * NPAD] = val
        idx2 = np.concatenate(
            [np.concatenate([fi[b0 + b], np.zeros(OUT2 - OUT, np.int64)])
             for b in range(NB)])
        meta_i = np.concatenate([_wrap16(idx1), _wrap16(idx2)], axis=1)

        gbt = np.zeros((128, NB * NVAL + NB * 2), np.float32)
        for b in range(NB):
            gbt[:, b * NVAL : (b + 1) * NVAL] = np.tile(gv[b0 + b], C)[None, :]
            for k in range(2):
                i = k * 128 + np.arange(128)
                valid = i < OUT
                gbt[valid, NB * NVAL + b * 2 + k] = fv[b0 + b][i[valid]]
        in_maps.append({"xt": xt, "meta_i": meta_i.astype(np.int16), "meta_f": gbt})
    return in_maps


def _identity_weight(weight):
    wref = np.zeros((C, C, 3, 3), np.float32)
    for c in range(C):
        wref[c, c, 1, 1] = 1.0
    return weight.shape == (C, C, 3, 3) and np.array_equal(weight, wref)


def _host_fallback(x, stride_h, stride_w, weight):
    """General-weight path (never hit with the module's fixed identity
    weight); numpy transcription of the reference for safety."""
    B = x.shape[0]
    dt = x.dtype
    ch = (stride_h + 1.0) * (H - 1) * 0.5
    cw = (stride_w + 1.0) * (W - 1) * 0.5
    offs = np.arange(3, dtype=dt) - 1.0
    ys = ch[:, :, None] + offs
    xs = cw[:, :, None] + offs

    def terms(coords, size):
        c0 = np.floor(coords)
        f = coords - c0
        i0 = c0.astype(np.int64)
        i1 = i0 + 1
        w0 = (1.0 - f) * ((i0 >= 0) & (i0 < size))
        w1 = f * ((i1 >= 0) & (i1 < size))
        return np.clip(i0, 0, size - 1), np.clip(i1, 0, size - 1), w0, w1

    yi0, yi1, wy0, wy1 = terms(ys, H)
    xi0, xi1, wx0, wx1 = terms(xs, W)
    out = np.zeros((B, C, OUT, OUT), dt)
    for b in range(B):
        row = (wy0[b][None, :, :, None] * x[b][:, yi0[b], :]
               + wy1[b][None, :, :, None] * x[b][:, yi1[b], :])
        samp = (wx0[b][None, None, None] * row[..., xi0[b]]
                + wx1[b][None, None, None] * row[..., xi1[b]])
        out[b] = np.einsum("ciujv,ocuv->oij", samp, weight)
    return out


def kernel(x, stride_h, stride_w, weight):
    x = np.asarray(x, np.float32)
    stride_h = np.asarray(stride_h, np.float32)
    stride_w = np.asarray(stride_w, np.float32)
    weight = np.asarray(weight, np.float32)
    if not _identity_weight(weight):
        return _host_fallback(x, stride_h, stride_w, weight)

    nc = _get_program()
    in_maps = make_in_maps(x, stride_h, stride_w)
    res = run_bass_kernel_spmd(nc, in_maps, core_ids=list(range(N_CORES)))
    out = np.empty((B_FULL, C, OUT, OUT), np.float32)
    for core in range(N_CORES):
        out[core * NB : (core + 1) * NB] = res.results[core]["out"]
    return out
